# revision 39
# baseline (speedup 1.0000x reference)
"""Trainium2 Bass kernel for DiffSelfAttention (B=1, T=2048, C=2048, 16 v-heads).

Sharding: tensor-parallel over heads across 8 NeuronCores. Core c owns
v-heads {2c, 2c+1} plus the matching q/k heads of both differential branches.
Each core computes its qkv slice, the attention for its 4 q/k head-pairs, the
differential + per-head RMSNorm, and a partial projection
y_c = out_c @ w_proj[rows_c]. The host sums the 8 partials (unshard step).

v4 design: single software-pipelined stream built around the ACT exp cadence
(the hard floor: 128 exps x ~1.34us = 171us).
  - tq blocks of 512 (4 blocks x 2 branches = 8 sweeps of 16 k-slabs).
    Each sweep computes BOTH v-heads per j via two concurrent 64-row
    matmuls (PE row-tiling: head 0 at partitions 0:64 -> tile (0,0),
    head 1 at 64:128 -> tile (64,0)), halving score PE time.
  - qkv is produced chunk-by-chunk: only k1/q1/v-half of t-chunk 0 precede
    the first sweep; the rest is a strict-FIFO filler queue drained into
    the exp-bound sweeps. ensure() force-drains through a needed group
    before any consumer is emitted (emission order IS dependency order in
    the tile framework - a consumer emitted before its producer races).
  - x is DMA'd in k-slab sub-chunks on two queue sets so production can
    start while later chunks stream (HBM limits x3 to ~40us).
  - exp'd scores [P, 2, 512] (both heads) per ACTIVATE; ones-column PV
    (129-wide moving) puts softmax denominators on the partition axis.
  - block b's combine/transpose/projection are fillers for block b+1's
    sweeps; only block 3's projection is tail-exposed.
  - PSUM: scores 2 slots x 2 banks + pv accumulators 3 banks (2 heads x 4
    q-tiles packed 3-per-bank) + 1 filler/proj bank = 8.
  - ACT does exp only once sweeps start (qkv copies ride DVE).
"""

import math
from collections import deque

import numpy as np

import concourse.bass as bass
import concourse.bacc as bacc
import concourse.mybir as mybir
import concourse.tile as tile

F32 = mybir.dt.float32
BF16 = mybir.dt.bfloat16

T = 2048
C = 2048
N_HEAD = 16
H_DIM = 64
D2 = 2 * H_DIM  # 128 (v-head dim, also the RMS group size)
LAMBDA_INIT = 0.8 - 0.6 * math.exp(-0.3)
SCALE = 1.0 / math.sqrt(H_DIM)
P = 128
KS = C // P  # 16 contraction slabs
TT = T // P  # 16 t-tiles
NCH = 512  # t-chunk width (DMA/production granularity AND tq-block width)
QS = 4  # tq tiles per block per head
NBLK = 4  # tq blocks of 512
N_CORES = 8

EXP = mybir.ActivationFunctionType.Exp
CPY = mybir.ActivationFunctionType.Copy
SQR = mybir.ActivationFunctionType.Square
MULT = mybir.AluOpType.mult
ADD = mybir.AluOpType.add


def build(lam: float) -> bass.Bass:
    nc = bacc.Bacc("TRN2", target_bir_lowering=False, debug=False)

    xb_d = nc.dram_tensor("xt", [P, 4, KS, NCH], BF16, kind="ExternalInput")
    wqk_d = nc.dram_tensor("wqk", [P, 4, KS, P], BF16, kind="ExternalInput")
    wv_d = nc.dram_tensor("wv", [P, KS, 2 * D2], BF16, kind="ExternalInput")
    wp_d = nc.dram_tensor("wp", [P, 2, T], BF16, kind="ExternalInput")
    id_d = nc.dram_tensor("ident", [P, P], BF16, kind="ExternalInput")
    y_d = nc.dram_tensor("y", [TT, P, T], BF16, kind="ExternalOutput")

    with tile.TileContext(nc) as tc:
        with tc.tile_pool(name="persist", bufs=1) as pp, \
             tc.tile_pool(name="etp", bufs=4) as etp, \
             tc.tile_pool(name="work", bufs=2) as wkp, \
             tc.tile_pool(name="ysp", bufs=4) as ysp, \
             tc.tile_pool(name="sc", bufs=2, space="PSUM") as scp, \
             tc.tile_pool(name="acc", bufs=1, space="PSUM") as accp, \
             tc.tile_pool(name="yp", bufs=1, space="PSUM") as ypp:

            xb = pp.tile([P, 4, KS, NCH], BF16)
            wqk = pp.tile([P, 4, KS, P], BF16)
            wv = pp.tile([P, KS, 2 * D2], BF16)
            wp = pp.tile([P, 2, T], BF16)
            qk = pp.tile([P, 4, T], BF16)  # m: q1|q2|k1|k2, [d, T] layout
            ident = pp.tile([P, P], BF16)
            vb = pp.tile([P, KS, 2, 130], BF16)  # [tk, kslab, vh, v|1|pad]

            # DMA issue engines matter: descriptor-gen rides the issuing
            # engine's FIFO and blocks everything behind it (incl. waiting
            # for queue slots). ACT gets only the 4 early x0 subs (its
            # copies/exps must not sit behind DMA issues); gpsimd (idle
            # engine, memset FIRST) carries the x bulk; sync carries
            # weights + x3 + the y writes.
            # DMA engines split bandwidth across ACTIVE queues, so a second
            # queue would steal from the critical x0/k1/q1/wv era. The sync
            # queue carries the early era (strict need-order); the gpsimd
            # queue is GATED on the k1 copy (a dummy gpsimd read of qk) so
            # its descriptor-gens fire only after the x0 era, then both
            # queues stream the late inputs in parallel.
            nc.gpsimd.memset(vb[:, :, :, D2:D2 + 1], 1.0)
            for h in range(4):
                nc.sync.dma_start(out=xb[:, 0, 4 * h:4 * h + 4],
                                  in_=xb_d[:, 0, 4 * h:4 * h + 4])
            nc.sync.dma_start(out=wqk[:, 2], in_=wqk_d[:, 2])  # k1
            nc.sync.dma_start(out=wqk[:, 0], in_=wqk_d[:, 0])  # q1
            nc.sync.dma_start(out=wv, in_=wv_d[:])
            nc.sync.dma_start(out=xb[:, 1, 8:16], in_=xb_d[:, 1, 8:16])
            nc.sync.dma_start(out=xb[:, 2, 8:16], in_=xb_d[:, 2, 8:16])
            nc.sync.dma_start(out=xb[:, 3, 8:16], in_=xb_d[:, 3, 8:16])
            nc.sync.dma_start(out=wqk[:, 1], in_=wqk_d[:, 1])  # q2
            nc.sync.dma_start(out=wp, in_=wp_d[:])

            # PE warm-up: dummy matmuls on never-written scratch (outputs
            # never read) keep the HAM activity window busy while the x0
            # sub-DMAs stream in, so the real production runs at 2.4 GHz
            wu_s = pp.tile([P, P], BF16)
            wu_m = pp.tile([P, NCH], BF16)
            nc.vector.memset(wu_s, 0.0)
            nc.vector.memset(wu_m, 0.0)
            wu_ps = scp.tile([P, NCH], F32, tag="s", name="warm")
            for _ in range(24):
                nc.tensor.matmul(wu_ps, wu_s, wu_m, start=True, stop=True)

            # ---------------- qkv production ----------------
            def prod_qk(n, m, pool, tag, act_copy):
                ps = pool.tile([P, NCH], F32, tag=tag, name=f"psq{n}{m}")
                for k in range(KS):
                    nc.tensor.matmul(
                        ps,
                        wqk[:, m, k, :],
                        xb[:, n, k, :],
                        start=(k == 0),
                        stop=(k == KS - 1),
                    )
                if act_copy:
                    nc.scalar.activation(qk[:, m, n * NCH:(n + 1) * NCH], ps,
                                         CPY)
                else:
                    nc.vector.tensor_copy(qk[:, m, n * NCH:(n + 1) * NCH], ps)

            def prod_v(n, t2, pool, tag, act_copy):
                g = 4 * n + t2
                ps = pool.tile([P, 2, D2], F32, tag=tag, name=f"psv{g}")
                for k in range(KS):
                    nc.tensor.matmul(
                        ps,
                        xb[:, n, k, t2 * P:(t2 + 1) * P],
                        wv[:, k, :],
                        start=(k == 0),
                        stop=(k == KS - 1),
                    )
                if act_copy:
                    nc.scalar.activation(vb[:, g, :, 0:D2], ps, CPY)
                else:
                    nc.vector.tensor_copy(vb[:, g, :, 0:D2], ps)

            # pre-sweep: chunk-0 k1 + q1 only, the two accumulation chains
            # interleaved per k-slab so they track the streaming x0
            # sub-DMAs (ACT copies - ACT idle until exp 0). v slabs ride
            # the filler queue (consumed from loop iter 1 on).
            psk = scp.tile([P, NCH], F32, tag="s", name="psk0")
            psq = scp.tile([P, NCH], F32, tag="s", name="psq0")
            for k in range(KS):
                st, sp = (k == 0), (k == KS - 1)
                nc.tensor.matmul(psk, wqk[:, 2, k, :], xb[:, 0, k, :],
                                 start=st, stop=sp)
                nc.tensor.matmul(psq, wqk[:, 0, k, :], xb[:, 0, k, :],
                                 start=st, stop=sp)
            nc.scalar.activation(qk[:, 2, 0:NCH], psk, CPY)
            nc.scalar.activation(qk[:, 0, 0:NCH], psq, CPY)
            # gpsimd DMA-queue gate: release the second input stream only
            # after the k1 copy lands (x0-era end). The scheduler reorders
            # independent work, so gate via REAL deps: scribble a sliver of
            # each DMA's dest (reads qk -> waits k1 copy; the DMA then has
            # a WAW dep on the scribble and overwrites it).
            for sliver, dst, src in (
                    (xb[:, 1, 0, 0:1], xb[:, 1, 0:8], xb_d[:, 1, 0:8]),
                    (xb[:, 2, 0, 0:1], xb[:, 2, 0:8], xb_d[:, 2, 0:8]),
                    (xb[:, 3, 0, 0:1], xb[:, 3, 0:8], xb_d[:, 3, 0:8]),
                    (wqk[:, 3, 0, 0:1], wqk[:, 3], wqk_d[:, 3]),
                    (ident[:, 0:1], ident, id_d[:])):
                nc.gpsimd.tensor_copy(sliver, qk[:, 2, 0:1])
                nc.gpsimd.dma_start(out=dst, in_=src)

            # ---- filler machinery.
            # prod groups in deadline order, each a contiguous run of
            # closures sharing one psum-bank tile. At most ONE entity owns
            # the shared "y" bank at a time (an open prod group or an open
            # proj tile); it must finish before another claims the bank.
            # ensure(key) finishes the owner then hard-drains the target
            # group before its consumer is emitted (emission order IS
            # dependency order in the tile framework).
            prod_groups = []  # {key, gate, est, cls: deque}
            owner = [None]    # open "y"-bank entity: deque of closures
            tp_fillers = deque()   # (est, blk, fn) - uses scores psum, exempt
            proj_fillers = deque()  # (blk, [closures...]) per proj tile
            # hard-drained groups copy PSUM->SBUF on ACT: the consumer is
            # stalling RIGHT NOW and the DVE queue may hold a whole
            # combine's backlog in front of the copy
            hard_drain = [False]

            def queue_prod_qk(n, m, gate):
                key = ("qk", n, m)
                box = {}

                def mk(k):
                    def f():
                        if k == 0:
                            box["ps"] = ypp.tile([P, NCH], F32, tag="y",
                                                 name=f"fq{n}{m}")
                        nc.tensor.matmul(
                            box["ps"],
                            wqk[:, m, k, :],
                            xb[:, n, k, :],
                            start=(k == 0),
                            stop=(k == KS - 1),
                        )
                        if k == KS - 1:
                            dst = qk[:, m, n * NCH:(n + 1) * NCH]
                            if hard_drain[0]:
                                nc.scalar.activation(dst, box["ps"], CPY)
                            else:
                                nc.vector.tensor_copy(dst, box["ps"])
                    return f

                prod_groups.append({"key": key, "gate": gate, "est": 220.0,
                                    "cls": deque(mk(k) for k in range(KS))})

            def queue_prod_v(n, t2, gate):
                g = 4 * n + t2
                key = ("v", g)
                box = {}

                def mk(k):
                    def f():
                        if k == 0:
                            box["ps"] = ypp.tile([P, 2, D2], F32, tag="y",
                                                 name=f"fv{g}")
                        nc.tensor.matmul(
                            box["ps"],
                            xb[:, n, k, t2 * P:(t2 + 1) * P],
                            wv[:, k, :],
                            start=(k == 0),
                            stop=(k == KS - 1),
                        )
                        if k == KS - 1:
                            dst = vb[:, g, :, 0:D2]
                            if hard_drain[0]:
                                nc.scalar.activation(dst, box["ps"], CPY)
                            else:
                                nc.vector.tensor_copy(dst, box["ps"])
                    return f

                prod_groups.append({"key": key, "gate": gate, "est": 115.0,
                                    "cls": deque(mk(k) for k in range(KS))})

            # deadline order; gates track x chunk DMA arrival in bj units
            # (bj time ~= 20us + ~1.5us*bj early; x1/x2/x3 at ~25/33/41us)
            queue_prod_v(0, 0, 0)
            queue_prod_v(0, 1, 0)
            queue_prod_v(0, 2, 0)
            queue_prod_v(0, 3, 0)
            queue_prod_qk(1, 2, 3)    # k1(1), deadline bj4
            for t2 in range(4):
                queue_prod_v(1, t2, 3)
            queue_prod_qk(2, 2, 8)    # k1(2), deadline bj8
            for t2 in range(4):
                queue_prod_v(2, t2, 8)
            queue_prod_qk(3, 2, 13)   # k1(3), deadline bj12
            for t2 in range(4):
                queue_prod_v(3, t2, 13)
            queue_prod_qk(0, 3, 0)    # k2(0), deadline bj16
            queue_prod_qk(0, 1, 0)    # q2(0), deadline bj16
            queue_prod_qk(1, 3, 3)    # k2(1), deadline bj20
            queue_prod_qk(2, 3, 8)    # k2(2), deadline bj24
            queue_prod_qk(3, 3, 13)   # k2(3), deadline bj28
            queue_prod_qk(1, 0, 3)    # q1(1), deadline bj32
            queue_prod_qk(1, 1, 8)    # q2(1), deadline bj48
            queue_prod_qk(2, 0, 8)    # q1(2), deadline bj64
            queue_prod_qk(2, 1, 13)   # q2(2), deadline bj80
            queue_prod_qk(3, 0, 13)   # q1(3), deadline bj96
            queue_prod_qk(3, 1, 13)   # q2(3), deadline bj112

            def finish_owner():
                o = owner[0]
                if o is not None:
                    while o:
                        o.popleft()()
                    owner[0] = None
                    prod_groups[:] = [g for g in prod_groups if g["cls"]]

            def ensure(key):
                g = next((x for x in prod_groups if x["key"] == key), None)
                if g is None or not g["cls"]:
                    return
                hard_drain[0] = True
                if owner[0] is not g["cls"]:
                    finish_owner()
                while g["cls"]:
                    g["cls"].popleft()()
                owner[0] = None
                if g in prod_groups:
                    prod_groups.remove(g)
                hard_drain[0] = False

            def ensure_k(br, chunk):
                ensure(("qk", chunk, 2 + br))

            def ensure_q(br, b):
                ensure(("qk", b, br))

            bj_box = {"bj": 0}

            owner_est = [220.0]
            rr = [0]  # round-robin: alternate prod vs tp/proj so neither
            # backlog survives to a block-top hard drain

            def inject(budget_ns, tp_ok=True):
                bj = bj_box["bj"]
                while budget_ns > 0:
                    o = owner[0]
                    if o is not None:
                        if o:
                            o.popleft()()
                            budget_ns -= owner_est[0]
                        if not o:
                            owner[0] = None
                            prod_groups[:] = [g for g in prod_groups
                                              if g["cls"]]
                        continue
                    g = next((x for x in prod_groups if x["gate"] <= bj),
                             None)
                    want_tp = (rr[0] % 2 == 1) and tp_ok and (
                        tp_fillers or proj_fillers)
                    rr[0] += 1
                    if g is not None and not want_tp:
                        owner[0] = g["cls"]
                        owner_est[0] = g["est"]
                        continue
                    if tp_fillers:
                        if not tp_ok:
                            return
                        est, _, f = tp_fillers.popleft()
                        f()
                        budget_ns -= est
                    elif proj_fillers:
                        if not tp_ok:
                            return
                        _, cls = proj_fillers.popleft()
                        owner[0] = deque(cls)
                        owner_est[0] = 480.0
                    elif g is not None:
                        owner[0] = g["cls"]
                        owner_est[0] = g["est"]
                    else:
                        return

            def drain_aged(b):
                # stale-tile guard: block b-2's o2/otT slots are reused at
                # block b, so everything reading them must be emitted first
                while tp_fillers and tp_fillers[0][1] <= b - 2:
                    tp_fillers.popleft()[2]()
                if proj_fillers and proj_fillers[0][0] <= b - 2:
                    finish_owner()
                    while proj_fillers and proj_fillers[0][0] <= b - 2:
                        _, cls = proj_fillers.popleft()
                        for c in cls:
                            c()

            # ---------------- attention sweeps ----------------
            # 8 pv accumulators (2 vh x 4 q-tiles) packed 3-per-bank.
            def get_accs(b, br):
                a = accp.tile([P, 3, 132], F32, tag="accA", name=f"acA{b}{br}")
                bb = accp.tile([P, 3, 132], F32, tag="accB", name=f"acB{b}{br}")
                c = accp.tile([P, 2, 132], F32, tag="accC", name=f"acC{b}{br}")
                # row r (vh*4+q) -> (tile, idx, first_of_bank, last_of_bank)
                return [(a, 0, True, False), (a, 1, False, False),
                        (a, 2, False, True), (bb, 0, True, False),
                        (bb, 1, False, False), (bb, 2, False, True),
                        (c, 0, True, False), (c, 1, False, True)]

            def sweep(b, br):
                # scores + exp + pv for branch br, tq block b, BOTH v-heads
                # packed per j (64-row PE tiles), pipelined one slab ahead.
                ensure_q(br, b)
                accs = get_accs(b, br)
                ets = [None] * KS
                cols = slice(b * NCH, (b + 1) * NCH)
                for j in range(KS + 1):
                    if j < KS:
                        ensure_k(br, j // 4)
                        ps = scp.tile([P, 2, NCH], F32, tag="s",
                                      name=f"sc{b}{br}{j}")
                        for vh in range(2):
                            rows = slice(vh * H_DIM, (vh + 1) * H_DIM)
                            nc.tensor.matmul(
                                ps[:, vh, :],
                                qk[rows, 2 + br, j * P:(j + 1) * P],
                                qk[rows, br, cols],
                                start=True,
                                stop=True,
                            )
                        et = etp.tile([P, 2, NCH], BF16, tag="e",
                                      name=f"et{b}{br}{j}")
                        nc.scalar.activation(et, ps, EXP, scale=SCALE)
                        ets[j] = et
                    if j > 0:
                        ensure(("v", j - 1))
                        first = b == 0 and br == 0
                        if j == 12 and not (b == NBLK - 1 and br == 1):
                            # pre-drain the NEXT sweep's q projection here,
                            # where the clock is warm and 4 more exps still
                            # cover the burst - not at the sweep boundary
                            nb, nbr = (b, 1) if br == 0 else (b + 1, 0)
                            ensure_q(nbr, nb)
                        # tp pairs read o2(b-1), produced by the previous
                        # combine: ~9us of serial DVE that overlaps this
                        # sweep's first ~7 iterations. Injecting a tp
                        # earlier would park the PE queue on the DVE chain
                        # and stall the exp stream.
                        inject(2500.0 if first else
                               (900.0 if j == 1 else 700.0),
                               tp_ok=(b + br > 0) and (br == 1 or j >= 8))
                        et = ets[j - 1]
                        for r in range(8):
                            vh, q = r // 4, r % 4
                            at, qi, first_b, last_b = accs[r]
                            nc.tensor.matmul(
                                at[:, qi, 0:129],
                                et[:, vh, q * P:(q + 1) * P],
                                vb[:, j - 1, vh, 0:129],
                                start=(j - 1 == 0) and first_b,
                                stop=(j - 1 == KS - 1) and last_b,
                            )
                        ets[j - 1] = None
                        bj_box["bj"] += 1
                return accs

            def save_accs(b, br, accs, tag):
                # rows 0:4 = vh0 q0..3, rows 4:8 = vh1 q0..3 (a|r cols)
                sb = wkp.tile([P, 8, 132], F32, tag=tag, name=f"{tag}{b}")
                nc.vector.tensor_copy(sb[:, 0:3, 0:129],
                                      accs[0][0][:, :, 0:129])
                nc.vector.tensor_copy(sb[:, 3:6, 0:129],
                                      accs[3][0][:, :, 0:129])
                nc.vector.tensor_copy(sb[:, 6:8, 0:129],
                                      accs[6][0][:, :, 0:129])
                return sb

            I32 = mybir.dt.int32
            SHR = mybir.AluOpType.logical_shift_right

            def make_rms_tail(msb, lns, rs):
                def rms_tail(qs):
                    # rs = rsqrt(msb) via bit-hack seed + 2 Newton steps
                    nc.vector.tensor_scalar(
                        out=lns[:, qs, :].bitcast(I32),
                        in0=msb[:, qs, :].bitcast(I32),
                        scalar1=1, scalar2=None, op0=SHR)
                    nc.vector.tensor_scalar(
                        out=rs[:, qs, :].bitcast(I32),
                        in0=lns[:, qs, :].bitcast(I32),
                        scalar1=-1, scalar2=0x5F3759DF, op0=MULT, op1=ADD)
                    for _ in range(2):
                        nc.vector.tensor_mul(lns[:, qs, :], rs[:, qs, :],
                                             rs[:, qs, :])
                        nc.vector.tensor_mul(lns[:, qs, :], lns[:, qs, :],
                                             msb[:, qs, :])
                        nc.vector.tensor_scalar(
                            out=lns[:, qs, :], in0=lns[:, qs, :],
                            scalar1=-0.5, scalar2=1.5, op0=MULT, op1=ADD)
                        nc.vector.tensor_mul(rs[:, qs, :], rs[:, qs, :],
                                             lns[:, qs, :])
                return rms_tail

            def combine(b, asb, bsb):
                # o' = a1*r2 - lam*a2*r1 (per-column rescale of the true o;
                # RMSNorm cancels it), then per-head RMS + bf16. Rows 0:8 =
                # (vh0 q0..3, vh1 q0..3).
                r1n = wkp.tile([P, 8, 1], F32, tag="r1n", name=f"r1n{b}")
                o12 = wkp.tile([P, 8, P], F32, tag="o12", name=f"o12{b}")
                sqs = wkp.tile([P, P], F32, tag="sqs", name=f"sqs{b}")
                msb = wkp.tile([P, 8, 1], F32, tag="msb", name=f"msb{b}")
                lns = wkp.tile([P, 8, 1], F32, tag="lns", name=f"lns{b}")
                rs = wkp.tile([P, 8, 1], F32, tag="rs", name=f"rs{b}")
                o2 = wkp.tile([P, 8, P], BF16, tag="o2", name=f"o2{b}")
                nc.vector.tensor_scalar_mul(r1n, asb[:, :, 128:129], -lam)
                for q in range(8):
                    nc.vector.tensor_scalar_mul(
                        o12[:, q, :], asb[:, q, 0:P], bsb[:, q, 128:129]
                    )
                    nc.vector.scalar_tensor_tensor(
                        o12[:, q, :], bsb[:, q, 0:P], r1n[:, q, :],
                        o12[:, q, :], op0=MULT, op1=ADD,
                    )
                    nc.vector.tensor_mul(sqs, o12[:, q, :], o12[:, q, :])
                    nc.vector.tensor_reduce(
                        msb[:, q, :], sqs, mybir.AxisListType.X, ADD
                    )
                make_rms_tail(msb, lns, rs)(slice(0, 8))
                for q in range(8):
                    nc.vector.tensor_scalar_mul(
                        o2[:, q, :], o12[:, q, :], rs[:, q, :]
                    )
                return o2

            def tail_block(b, asb, bsb, otT, tail_rot):
                # final block: pair-major rows (row 2q+h = v-head h, tile q)
                # so the rsqrt batches split per pair-pair; o2/tp/proj for
                # pairs 0,1 overlap the second rsqrt batch. DVE/ACT split
                # throughout so the engines pipeline.
                r1n = wkp.tile([P, 8, 1], F32, tag="r1n", name="r1nT")
                o12 = wkp.tile([P, 8, P], F32, tag="o12", name="o12T")
                sqs = wkp.tile([P, P], F32, tag="sqs", name="sqsT")
                msb = wkp.tile([P, 8, 1], F32, tag="msb", name="msbT")
                lns = wkp.tile([P, 8, 1], F32, tag="lns", name="lnsT")
                rs = wkp.tile([P, 8, 1], F32, tag="rs", name="rsT")
                o2 = wkp.tile([P, 8, P], BF16, tag="o2", name="o2T")
                nc.vector.tensor_scalar_mul(r1n, asb[:, :, 128:129], -lam)
                for rr in range(8):
                    q, h = rr // 2, rr % 2
                    r = q + 4 * h
                    nc.vector.tensor_scalar_mul(
                        o12[:, rr, :], asb[:, r, 0:P], bsb[:, r, 128:129]
                    )
                    nc.vector.scalar_tensor_tensor(
                        o12[:, rr, :], bsb[:, r, 0:P], r1n[:, r, :],
                        o12[:, rr, :], op0=MULT, op1=ADD,
                    )
                    nc.scalar.activation(sqs, o12[:, rr, :], SQR,
                                         accum_out=msb[:, rr, :])
                rms = make_rms_tail(msb, lns, rs)
                for half in range(2):
                    rms(slice(4 * half, 4 * half + 4))
                    for q in (2 * half, 2 * half + 1):
                        nc.vector.tensor_scalar_mul(
                            o2[:, 2 * q, :], o12[:, 2 * q, :],
                            rs[:, 2 * q, :]
                        )
                        nc.scalar.activation(o2[:, 2 * q + 1, :],
                                             o12[:, 2 * q + 1, :],
                                             CPY, scale=rs[:, 2 * q + 1, :])
                        emit_tp(o2, 2 * q, otT, 0, q, psum_src=(scp, "s"),
                                act_copy=False)
                        emit_tp(o2, 2 * q + 1, otT, 1, q,
                                psum_src=(scp, "s"), act_copy=True)
                        for c in proj_tile_closures(otT, b * QS + q,
                                                    tail_rot[q % 4],
                                                    split_copies=True):
                            c()

            def emit_tp(o2, r, otT, vh, q, psum_src=None, act_copy=False):
                # transpose o2 row r ([tq,d2] -> [d2,tq]) via PE
                pool, tag = psum_src if psum_src else (scp, "s")
                pt = pool.tile([P, P], BF16, tag=tag, name=f"tp{vh}{q}")
                nc.tensor.transpose(pt, o2[:, r, :], ident)
                if act_copy:
                    nc.scalar.activation(otT[:, vh, q, :], pt, CPY)
                else:
                    nc.vector.tensor_copy(otT[:, vh, q, :], pt)

            def queue_tp_pairs(o2, otT, b):
                # pairs (vh0 q, vh1 q): keeps the 2-slot scores rotation
                # aligned AND completes one proj tile's deps per pair
                for q in range(QS):
                    def f(q=q):
                        emit_tp(o2, q, otT, 0, q)
                        emit_tp(o2, 4 + q, otT, 1, q)
                    tp_fillers.append((650.0, b, f))

            # -------- output projection for one 128-row tq tile ----------
            def proj_tile_closures(otT, t, psum_src, split_copies):
                q = t % QS
                pool, tag = psum_src
                box = {}
                cl = []

                def c_vh0(p):
                    def f():
                        if p == 0:
                            box["yp"] = pool.tile([P, 2, 256], F32, tag=tag,
                                                  name=f"yt{t}")
                            box["ys"] = ysp.tile([P, 4, 2, 256], BF16,
                                                 tag="ysb", name=f"ys{t}")
                        yp = box["yp"]
                        for r in range(2):
                            nc.tensor.matmul(
                                yp[:, r, :],
                                otT[:, 0, q, :],
                                wp[:, 0, 512 * p + 256 * r:
                                   512 * p + 256 * (r + 1)],
                                start=(r == 0),
                                stop=False,
                            )
                    return f

                def c_vh1(p):
                    def f():
                        yp = box["yp"]
                        for r in range(2):
                            nc.tensor.matmul(
                                yp[:, r, :],
                                otT[:, 1, q, :],
                                wp[:, 1, 512 * p + 256 * r:
                                   512 * p + 256 * (r + 1)],
                                start=False,
                                stop=(r == 1),
                            )
                        if split_copies:
                            nc.vector.tensor_copy(box["ys"][:, p, 0, :],
                                                  yp[:, 0, :])
                            nc.scalar.activation(box["ys"][:, p, 1, :],
                                                 yp[:, 1, :], CPY)
                            if p == 1:  # stream out the finished half
                                nc.sync.dma_start(
                                    out=y_d[t, :, 0:1024],
                                    in_=box["ys"][:, 0:2])
                        else:
                            nc.vector.tensor_copy(box["ys"][:, p, :, :], yp)
                    return f

                def c_dma():
                    if split_copies:
                        nc.sync.dma_start(out=y_d[t, :, 1024:2048],
                                          in_=box["ys"][:, 2:4])
                    else:
                        nc.sync.dma_start(out=y_d[t], in_=box["ys"])

                for p in range(4):
                    cl.append(c_vh0(p))
                    cl.append(c_vh1(p))
                cl.append(c_dma)
                return cl

            # ---------------- blocks ----------------
            tail_rot = [(ypp, "y"), (accp, "accA"), (accp, "accB"),
                        (accp, "accC")]
            warm_tiles = []  # block-2 proj tiles held back to keep the PE
            # (and HAM) warm during the tail combine's DVE/ACT phase
            for b in range(NBLK):
                last = b == NBLK - 1
                drain_aged(b)
                otT = wkp.tile([P, 2, QS, P], BF16, tag="otT", name=f"otT{b}")
                accs0 = sweep(b, 0)
                asb = save_accs(b, 0, accs0, "asb")
                accs1 = sweep(b, 1)
                bsb = save_accs(b, 1, accs1, "bsb")
                if not last:
                    o2 = combine(b, asb, bsb)
                    queue_tp_pairs(o2, otT, b)
                    for t in range(QS):
                        cls = proj_tile_closures(otT, b * QS + t,
                                                 (ypp, "y"),
                                                 split_copies=False)
                        if b == NBLK - 2 and t >= 2:
                            warm_tiles.append(cls)
                        else:
                            proj_fillers.append((b, cls))
                else:
                    # drain leftovers + held-back tiles first (PE chews
                    # them while DVE/ACT run the tail combine), then the
                    # per-pair pipelined tail.
                    finish_owner()
                    for g in list(prod_groups):
                        ensure(g["key"])
                    while tp_fillers:
                        tp_fillers.popleft()[2]()
                    while proj_fillers:
                        _, cls = proj_fillers.popleft()
                        for c in cls:
                            c()
                    for cls in warm_tiles:
                        for c in cls:
                            c()
                    tail_block(b, asb, bsb, otT, tail_rot)
    nc.finalize()
    return nc


def _core_inputs(x, w_qkv, w_proj, rms_scale):
    """Host-side shard prep: per-core bf16 weight slices + replicated x^T."""
    bf = mybir.dt.np(BF16)
    ident = np.ascontiguousarray(np.eye(P, dtype=np.float32).astype(bf))
    xt = x.reshape(T, C).T  # [C, T]
    xtr = np.ascontiguousarray(
        xt.reshape(KS, P, 4, NCH).transpose(1, 2, 0, 3).astype(bf)
    )
    sv = np.tile(
        rms_scale.astype(np.float32) * np.float32(1.0 - LAMBDA_INIT)
        * np.float32(math.sqrt(D2)), 2
    )  # [256]; sqrt(D2) because the kernel's rsqrt takes the SUM of squares
    maps = []
    for c in range(N_CORES):
        cols = [
            w_qkv[:, 0 * 1024 + c * P:0 * 1024 + (c + 1) * P],  # q1 heads 2c,2c+1
            w_qkv[:, 1 * 1024 + c * P:1 * 1024 + (c + 1) * P],  # q2
            w_qkv[:, 2 * 1024 + c * P:2 * 1024 + (c + 1) * P],  # k1
            w_qkv[:, 3 * 1024 + c * P:3 * 1024 + (c + 1) * P],  # k2
        ]
        wqk = np.stack(cols, axis=0)  # [4, C, 128]
        wqk = np.ascontiguousarray(
            wqk.reshape(4, KS, P, P).transpose(2, 0, 1, 3).astype(bf)
        )
        wv = w_qkv[:, 2 * C + c * 2 * D2:2 * C + (c + 1) * 2 * D2]  # [C, 256]
        wv = np.ascontiguousarray(
            wv.reshape(KS, P, 2 * D2).transpose(1, 0, 2).astype(bf)
        )
        wp = w_proj[c * 2 * D2:(c + 1) * 2 * D2, :] * sv[:, None]  # [256, T]
        wp = np.ascontiguousarray(
            wp.reshape(2, P, T).transpose(1, 0, 2).astype(bf)
        )
        maps.append({"xt": xtr, "wqk": wqk, "wv": wv, "wp": wp, "ident": ident})
    return maps


def kernel(x, w_qkv, w_proj, lambda_q1, lambda_k1, lambda_q2, lambda_k2, rms_scale):
    from concourse.bass_utils import run_bass_kernel_spmd

    x = np.asarray(x, dtype=np.float32)
    w_qkv = np.asarray(w_qkv, dtype=np.float32)
    w_proj = np.asarray(w_proj, dtype=np.float32)
    rms_scale = np.asarray(rms_scale, dtype=np.float32)
    lam1 = np.exp(np.sum(np.asarray(lambda_q1) * np.asarray(lambda_k1), dtype=np.float32))
    lam2 = np.exp(np.sum(np.asarray(lambda_q2) * np.asarray(lambda_k2), dtype=np.float32))
    lam = float(lam1 - lam2 + LAMBDA_INIT)

    nc = build(lam)
    in_maps = _core_inputs(x, w_qkv, w_proj, rms_scale)
    res = run_bass_kernel_spmd(nc, in_maps, core_ids=list(range(N_CORES)))
    y = np.zeros((TT, P, T), np.float32)
    for rmap in res.results:
        y += np.asarray(rmap["y"], np.float32)
    return y.reshape(1, T, C)


# revision 41
# speedup vs baseline: 1.1720x; 1.1720x over previous
"""Trainium2 Bass kernel for DiffSelfAttention (B=1, T=2048, C=2048, 16 v-heads).

Sharding: tensor-parallel over heads across 8 NeuronCores. Core c owns
v-heads {2c, 2c+1} plus the matching q/k heads of both differential branches.
Each core computes its qkv slice, the attention for its 4 q/k head-pairs, the
differential + per-head RMSNorm, and a partial projection
y_c = out_c @ w_proj[rows_c]. The host sums the 8 partials (unshard step).

v4 design: single software-pipelined stream built around the ACT exp cadence
(the hard floor: 128 exps x ~1.34us = 171us).
  - tq blocks of 512 (4 blocks x 2 branches = 8 sweeps of 16 k-slabs).
    Each sweep computes BOTH v-heads per j via two concurrent 64-row
    matmuls (PE row-tiling: head 0 at partitions 0:64 -> tile (0,0),
    head 1 at 64:128 -> tile (64,0)), halving score PE time.
  - qkv is produced chunk-by-chunk: only k1/q1/v-half of t-chunk 0 precede
    the first sweep; the rest is a strict-FIFO filler queue drained into
    the exp-bound sweeps. ensure() force-drains through a needed group
    before any consumer is emitted (emission order IS dependency order in
    the tile framework - a consumer emitted before its producer races).
  - x is DMA'd in k-slab sub-chunks on two queue sets so production can
    start while later chunks stream (HBM limits x3 to ~40us).
  - exp'd scores [P, 2, 512] (both heads) per ACTIVATE; ones-column PV
    (129-wide moving) puts softmax denominators on the partition axis.
  - block b's combine/transpose/projection are fillers for block b+1's
    sweeps; only block 3's projection is tail-exposed.
  - PSUM: scores 2 slots x 2 banks + pv accumulators 3 banks (2 heads x 4
    q-tiles packed 3-per-bank) + 1 filler/proj bank = 8.
  - ACT does exp only once sweeps start (qkv copies ride DVE).
"""

import math
from collections import deque

import numpy as np

import concourse.bass as bass
import concourse.bacc as bacc
import concourse.mybir as mybir
import concourse.tile as tile

F32 = mybir.dt.float32
BF16 = mybir.dt.bfloat16

T = 2048
C = 2048
N_HEAD = 16
H_DIM = 64
D2 = 2 * H_DIM  # 128 (v-head dim, also the RMS group size)
LAMBDA_INIT = 0.8 - 0.6 * math.exp(-0.3)
SCALE = 1.0 / math.sqrt(H_DIM)
P = 128
KS = C // P  # 16 contraction slabs
TT = T // P  # 16 t-tiles
NCH = 512  # t-chunk width (DMA/production granularity AND tq-block width)
QS = 4  # tq tiles per block per head
NBLK = 4  # tq blocks of 512
N_CORES = 8

EXP = mybir.ActivationFunctionType.Exp
CPY = mybir.ActivationFunctionType.Copy
SQR = mybir.ActivationFunctionType.Square
MULT = mybir.AluOpType.mult
ADD = mybir.AluOpType.add


def build(lam: float) -> bass.Bass:
    nc = bacc.Bacc("TRN2", target_bir_lowering=False, debug=False)

    xb_d = nc.dram_tensor("xt", [P, 4, KS, NCH], BF16, kind="ExternalInput")
    wqk_d = nc.dram_tensor("wqk", [P, 4, KS, P], BF16, kind="ExternalInput")
    wv_d = nc.dram_tensor("wv", [P, KS, 2 * D2], BF16, kind="ExternalInput")
    wp_d = nc.dram_tensor("wp", [P, 2, T], BF16, kind="ExternalInput")
    id_d = nc.dram_tensor("ident", [P, P], BF16, kind="ExternalInput")
    y_d = nc.dram_tensor("y", [TT, P, T], BF16, kind="ExternalOutput")

    with tile.TileContext(nc) as tc:
        with tc.tile_pool(name="persist", bufs=1) as pp, \
             tc.tile_pool(name="etp", bufs=4) as etp, \
             tc.tile_pool(name="work", bufs=2) as wkp, \
             tc.tile_pool(name="ysp", bufs=4) as ysp, \
             tc.tile_pool(name="sc", bufs=2, space="PSUM") as scp, \
             tc.tile_pool(name="acc", bufs=1, space="PSUM") as accp, \
             tc.tile_pool(name="yp", bufs=1, space="PSUM") as ypp:

            xb = pp.tile([P, 4, KS, NCH], BF16)
            wqk = pp.tile([P, 4, KS, P], BF16)
            wv = pp.tile([P, KS, 2 * D2], BF16)
            wp = pp.tile([P, 2, T], BF16)
            qk = pp.tile([P, 4, T], BF16)  # m: q1|q2|k1|k2, [d, T] layout
            ident = pp.tile([P, P], BF16)
            vb = pp.tile([P, KS, 2, 130], BF16)  # [tk, kslab, vh, v|1|pad]

            # DMA issue engines matter: descriptor-gen rides the issuing
            # engine's FIFO and blocks everything behind it (incl. waiting
            # for queue slots). ACT gets only the 4 early x0 subs (its
            # copies/exps must not sit behind DMA issues); gpsimd (idle
            # engine, memset FIRST) carries the x bulk; sync carries
            # weights + x3 + the y writes.
            # DMA engines split bandwidth across ACTIVE queues, so a second
            # queue would steal from the critical x0/k1/q1/wv era. The sync
            # queue carries the early era (strict need-order); the gpsimd
            # queue is GATED on the k1 copy (a dummy gpsimd read of qk) so
            # its descriptor-gens fire only after the x0 era, then both
            # queues stream the late inputs in parallel.
            nc.gpsimd.memset(vb[:, :, :, D2:D2 + 1], 1.0)
            for h in range(4):
                nc.sync.dma_start(out=xb[:, 0, 4 * h:4 * h + 4],
                                  in_=xb_d[:, 0, 4 * h:4 * h + 4])
            nc.sync.dma_start(out=wqk[:, 2], in_=wqk_d[:, 2])  # k1
            nc.sync.dma_start(out=wqk[:, 0], in_=wqk_d[:, 0])  # q1
            nc.sync.dma_start(out=wv, in_=wv_d[:])
            nc.sync.dma_start(out=xb[:, 1, 0:8], in_=xb_d[:, 1, 0:8])
            nc.sync.dma_start(out=xb[:, 1, 8:16], in_=xb_d[:, 1, 8:16])
            nc.sync.dma_start(out=xb[:, 2, 0:8], in_=xb_d[:, 2, 0:8])
            nc.sync.dma_start(out=xb[:, 2, 8:16], in_=xb_d[:, 2, 8:16])
            nc.sync.dma_start(out=xb[:, 3, 0:8], in_=xb_d[:, 3, 0:8])
            nc.sync.dma_start(out=xb[:, 3, 8:16], in_=xb_d[:, 3, 8:16])
            nc.sync.dma_start(out=wqk[:, 3], in_=wqk_d[:, 3])  # k2
            nc.sync.dma_start(out=wqk[:, 1], in_=wqk_d[:, 1])  # q2
            nc.sync.dma_start(out=ident, in_=id_d[:])
            nc.sync.dma_start(out=wp, in_=wp_d[:])

            # PE warm-up: dummy matmuls on never-written scratch (outputs
            # never read) keep the HAM activity window busy while the x0
            # sub-DMAs stream in, so the real production runs at 2.4 GHz
            wu_s = pp.tile([P, P], BF16)
            wu_m = pp.tile([P, NCH], BF16)
            nc.vector.memset(wu_s, 0.0)
            nc.vector.memset(wu_m, 0.0)
            wu_ps = scp.tile([P, NCH], F32, tag="s", name="warm")
            for _ in range(24):
                nc.tensor.matmul(wu_ps, wu_s, wu_m, start=True, stop=True)

            # ---------------- qkv production ----------------
            def prod_qk(n, m, pool, tag, act_copy):
                ps = pool.tile([P, NCH], F32, tag=tag, name=f"psq{n}{m}")
                for k in range(KS):
                    nc.tensor.matmul(
                        ps,
                        wqk[:, m, k, :],
                        xb[:, n, k, :],
                        start=(k == 0),
                        stop=(k == KS - 1),
                    )
                if act_copy:
                    nc.scalar.activation(qk[:, m, n * NCH:(n + 1) * NCH], ps,
                                         CPY)
                else:
                    nc.vector.tensor_copy(qk[:, m, n * NCH:(n + 1) * NCH], ps)

            def prod_v(n, t2, pool, tag, act_copy):
                g = 4 * n + t2
                ps = pool.tile([P, 2, D2], F32, tag=tag, name=f"psv{g}")
                for k in range(KS):
                    nc.tensor.matmul(
                        ps,
                        xb[:, n, k, t2 * P:(t2 + 1) * P],
                        wv[:, k, :],
                        start=(k == 0),
                        stop=(k == KS - 1),
                    )
                if act_copy:
                    nc.scalar.activation(vb[:, g, :, 0:D2], ps, CPY)
                else:
                    nc.vector.tensor_copy(vb[:, g, :, 0:D2], ps)

            # pre-sweep: chunk-0 k1 + q1 only, the two accumulation chains
            # interleaved per k-slab so they track the streaming x0
            # sub-DMAs (ACT copies - ACT idle until exp 0). v slabs ride
            # the filler queue (consumed from loop iter 1 on).
            psk = scp.tile([P, NCH], F32, tag="s", name="psk0")
            psq = scp.tile([P, NCH], F32, tag="s", name="psq0")
            for k in range(KS):
                st, sp = (k == 0), (k == KS - 1)
                nc.tensor.matmul(psk, wqk[:, 2, k, :], xb[:, 0, k, :],
                                 start=st, stop=sp)
                nc.tensor.matmul(psq, wqk[:, 0, k, :], xb[:, 0, k, :],
                                 start=st, stop=sp)
            nc.scalar.activation(qk[:, 2, 0:NCH], psk, CPY)
            nc.scalar.activation(qk[:, 0, 0:NCH], psq, CPY)


            # ---- filler machinery.
            # prod groups in deadline order, each a contiguous run of
            # closures sharing one psum-bank tile. At most ONE entity owns
            # the shared "y" bank at a time (an open prod group or an open
            # proj tile); it must finish before another claims the bank.
            # ensure(key) finishes the owner then hard-drains the target
            # group before its consumer is emitted (emission order IS
            # dependency order in the tile framework).
            prod_groups = []  # {key, gate, est, cls: deque}
            owner = [None]    # open "y"-bank entity: deque of closures
            tp_fillers = deque()   # (est, blk, fn) - uses scores psum, exempt
            proj_fillers = deque()  # (blk, [closures...]) per proj tile
            # hard-drained groups copy PSUM->SBUF on ACT: the consumer is
            # stalling RIGHT NOW and the DVE queue may hold a whole
            # combine's backlog in front of the copy
            hard_drain = [False]

            def queue_prod_qk(n, m, gate):
                key = ("qk", n, m)
                box = {}

                def mk(k):
                    def f():
                        if k == 0:
                            box["ps"] = ypp.tile([P, NCH], F32, tag="y",
                                                 name=f"fq{n}{m}")
                        nc.tensor.matmul(
                            box["ps"],
                            wqk[:, m, k, :],
                            xb[:, n, k, :],
                            start=(k == 0),
                            stop=(k == KS - 1),
                        )
                        if k == KS - 1:
                            dst = qk[:, m, n * NCH:(n + 1) * NCH]
                            if hard_drain[0]:
                                nc.scalar.activation(dst, box["ps"], CPY)
                            else:
                                nc.vector.tensor_copy(dst, box["ps"])
                    return f

                prod_groups.append({"key": key, "gate": gate, "est": 220.0,
                                    "cls": deque(mk(k) for k in range(KS))})

            def queue_prod_v(n, t2, gate):
                g = 4 * n + t2
                key = ("v", g)
                box = {}

                def mk(k):
                    def f():
                        if k == 0:
                            box["ps"] = ypp.tile([P, 2, D2], F32, tag="y",
                                                 name=f"fv{g}")
                        nc.tensor.matmul(
                            box["ps"],
                            xb[:, n, k, t2 * P:(t2 + 1) * P],
                            wv[:, k, :],
                            start=(k == 0),
                            stop=(k == KS - 1),
                        )
                        if k == KS - 1:
                            dst = vb[:, g, :, 0:D2]
                            if hard_drain[0]:
                                nc.scalar.activation(dst, box["ps"], CPY)
                            else:
                                nc.vector.tensor_copy(dst, box["ps"])
                    return f

                prod_groups.append({"key": key, "gate": gate, "est": 115.0,
                                    "cls": deque(mk(k) for k in range(KS))})

            # deadline order; gates track x chunk DMA arrival in bj units
            # (bj time ~= 20us + ~1.5us*bj early; x1/x2/x3 at ~25/33/41us)
            queue_prod_v(0, 0, 0)
            queue_prod_v(0, 1, 0)
            queue_prod_v(0, 2, 0)
            queue_prod_v(0, 3, 0)
            queue_prod_qk(1, 2, 3)    # k1(1), deadline bj4
            for t2 in range(4):
                queue_prod_v(1, t2, 3)
            queue_prod_qk(2, 2, 8)    # k1(2), deadline bj8
            for t2 in range(4):
                queue_prod_v(2, t2, 8)
            queue_prod_qk(3, 2, 13)   # k1(3), deadline bj12
            for t2 in range(4):
                queue_prod_v(3, t2, 13)
            queue_prod_qk(0, 3, 0)    # k2(0), deadline bj16
            queue_prod_qk(0, 1, 0)    # q2(0), deadline bj16
            queue_prod_qk(1, 3, 3)    # k2(1), deadline bj20
            queue_prod_qk(2, 3, 8)    # k2(2), deadline bj24
            queue_prod_qk(3, 3, 13)   # k2(3), deadline bj28
            queue_prod_qk(1, 0, 3)    # q1(1), deadline bj32
            queue_prod_qk(1, 1, 8)    # q2(1), deadline bj48
            queue_prod_qk(2, 0, 8)    # q1(2), deadline bj64
            queue_prod_qk(2, 1, 13)   # q2(2), deadline bj80
            queue_prod_qk(3, 0, 13)   # q1(3), deadline bj96
            queue_prod_qk(3, 1, 13)   # q2(3), deadline bj112

            def finish_owner():
                o = owner[0]
                if o is not None:
                    while o:
                        o.popleft()()
                    owner[0] = None
                    prod_groups[:] = [g for g in prod_groups if g["cls"]]

            def ensure(key):
                g = next((x for x in prod_groups if x["key"] == key), None)
                if g is None or not g["cls"]:
                    return
                hard_drain[0] = True
                if owner[0] is not g["cls"]:
                    finish_owner()
                while g["cls"]:
                    g["cls"].popleft()()
                owner[0] = None
                if g in prod_groups:
                    prod_groups.remove(g)
                hard_drain[0] = False

            def ensure_k(br, chunk):
                ensure(("qk", chunk, 2 + br))

            def ensure_q(br, b):
                ensure(("qk", b, br))

            bj_box = {"bj": 0}

            owner_est = [220.0]
            rr = [0]  # round-robin: alternate prod vs tp/proj so neither
            # backlog survives to a block-top hard drain

            def inject(budget_ns, tp_ok=True):
                bj = bj_box["bj"]
                while budget_ns > 0:
                    o = owner[0]
                    if o is not None:
                        if o:
                            o.popleft()()
                            budget_ns -= owner_est[0]
                        if not o:
                            owner[0] = None
                            prod_groups[:] = [g for g in prod_groups
                                              if g["cls"]]
                        continue
                    g = next((x for x in prod_groups if x["gate"] <= bj),
                             None)
                    want_tp = (rr[0] % 2 == 1) and tp_ok and (
                        tp_fillers or proj_fillers)
                    rr[0] += 1
                    if g is not None and not want_tp:
                        owner[0] = g["cls"]
                        owner_est[0] = g["est"]
                        continue
                    if tp_fillers:
                        if not tp_ok:
                            return
                        est, _, f = tp_fillers.popleft()
                        f()
                        budget_ns -= est
                    elif proj_fillers:
                        if not tp_ok:
                            return
                        _, cls = proj_fillers.popleft()
                        owner[0] = deque(cls)
                        owner_est[0] = 480.0
                    elif g is not None:
                        owner[0] = g["cls"]
                        owner_est[0] = g["est"]
                    else:
                        return

            def drain_aged(b):
                # stale-tile guard: block b-2's o2/otT slots are reused at
                # block b, so everything reading them must be emitted first
                while tp_fillers and tp_fillers[0][1] <= b - 2:
                    tp_fillers.popleft()[2]()
                if proj_fillers and proj_fillers[0][0] <= b - 2:
                    finish_owner()
                    while proj_fillers and proj_fillers[0][0] <= b - 2:
                        _, cls = proj_fillers.popleft()
                        for c in cls:
                            c()

            # ---------------- attention sweeps ----------------
            # 8 pv accumulators (2 vh x 4 q-tiles) packed 3-per-bank.
            def get_accs(b, br):
                a = accp.tile([P, 3, 132], F32, tag="accA", name=f"acA{b}{br}")
                bb = accp.tile([P, 3, 132], F32, tag="accB", name=f"acB{b}{br}")
                c = accp.tile([P, 2, 132], F32, tag="accC", name=f"acC{b}{br}")
                # row r (vh*4+q) -> (tile, idx, first_of_bank, last_of_bank)
                return [(a, 0, True, False), (a, 1, False, False),
                        (a, 2, False, True), (bb, 0, True, False),
                        (bb, 1, False, False), (bb, 2, False, True),
                        (c, 0, True, False), (c, 1, False, True)]

            def sweep(b, br):
                # scores + exp + pv for branch br, tq block b, BOTH v-heads
                # packed per j (64-row PE tiles), pipelined one slab ahead.
                ensure_q(br, b)
                accs = get_accs(b, br)
                ets = [None] * KS
                cols = slice(b * NCH, (b + 1) * NCH)
                for j in range(KS + 1):
                    if j < KS:
                        ensure_k(br, j // 4)
                        ps = scp.tile([P, 2, NCH], F32, tag="s",
                                      name=f"sc{b}{br}{j}")
                        for vh in range(2):
                            rows = slice(vh * H_DIM, (vh + 1) * H_DIM)
                            nc.tensor.matmul(
                                ps[:, vh, :],
                                qk[rows, 2 + br, j * P:(j + 1) * P],
                                qk[rows, br, cols],
                                start=True,
                                stop=True,
                            )
                        et = etp.tile([P, 2, NCH], BF16, tag="e",
                                      name=f"et{b}{br}{j}")
                        nc.scalar.activation(et, ps, EXP, scale=SCALE)
                        ets[j] = et
                    if j > 0:
                        ensure(("v", j - 1))
                        first = b == 0 and br == 0
                        if j == 12 and not (b == NBLK - 1 and br == 1):
                            # pre-drain the NEXT sweep's q projection here,
                            # where the clock is warm and 4 more exps still
                            # cover the burst - not at the sweep boundary
                            nb, nbr = (b, 1) if br == 0 else (b + 1, 0)
                            ensure_q(nbr, nb)
                        # tp pairs read o2(b-1), produced by the previous
                        # combine: ~9us of serial DVE that overlaps this
                        # sweep's first ~7 iterations. Injecting a tp
                        # earlier would park the PE queue on the DVE chain
                        # and stall the exp stream.
                        inject(2500.0 if first else
                               (900.0 if j == 1 else 700.0),
                               tp_ok=(b + br > 0) and (br == 1 or j >= 8))
                        et = ets[j - 1]
                        for r in range(8):
                            vh, q = r // 4, r % 4
                            at, qi, first_b, last_b = accs[r]
                            nc.tensor.matmul(
                                at[:, qi, 0:129],
                                et[:, vh, q * P:(q + 1) * P],
                                vb[:, j - 1, vh, 0:129],
                                start=(j - 1 == 0) and first_b,
                                stop=(j - 1 == KS - 1) and last_b,
                            )
                        ets[j - 1] = None
                        bj_box["bj"] += 1
                return accs

            def save_accs(b, br, accs, tag):
                # rows 0:4 = vh0 q0..3, rows 4:8 = vh1 q0..3 (a|r cols)
                sb = wkp.tile([P, 8, 132], F32, tag=tag, name=f"{tag}{b}")
                nc.vector.tensor_copy(sb[:, 0:3, 0:129],
                                      accs[0][0][:, :, 0:129])
                nc.vector.tensor_copy(sb[:, 3:6, 0:129],
                                      accs[3][0][:, :, 0:129])
                nc.vector.tensor_copy(sb[:, 6:8, 0:129],
                                      accs[6][0][:, :, 0:129])
                return sb

            I32 = mybir.dt.int32
            SHR = mybir.AluOpType.logical_shift_right

            def make_rms_tail(msb, lns, rs):
                def rms_tail(qs):
                    # rs = rsqrt(msb) via bit-hack seed + 2 Newton steps
                    nc.vector.tensor_scalar(
                        out=lns[:, qs, :].bitcast(I32),
                        in0=msb[:, qs, :].bitcast(I32),
                        scalar1=1, scalar2=None, op0=SHR)
                    nc.vector.tensor_scalar(
                        out=rs[:, qs, :].bitcast(I32),
                        in0=lns[:, qs, :].bitcast(I32),
                        scalar1=-1, scalar2=0x5F3759DF, op0=MULT, op1=ADD)
                    for _ in range(2):
                        nc.vector.tensor_mul(lns[:, qs, :], rs[:, qs, :],
                                             rs[:, qs, :])
                        nc.vector.tensor_mul(lns[:, qs, :], lns[:, qs, :],
                                             msb[:, qs, :])
                        nc.vector.tensor_scalar(
                            out=lns[:, qs, :], in0=lns[:, qs, :],
                            scalar1=-0.5, scalar2=1.5, op0=MULT, op1=ADD)
                        nc.vector.tensor_mul(rs[:, qs, :], rs[:, qs, :],
                                             lns[:, qs, :])
                return rms_tail

            def combine(b, asb, bsb):
                # o' = a1*r2 - lam*a2*r1 (per-column rescale of the true o;
                # RMSNorm cancels it), then per-head RMS + bf16. Rows 0:8 =
                # (vh0 q0..3, vh1 q0..3).
                r1n = wkp.tile([P, 8, 1], F32, tag="r1n", name=f"r1n{b}")
                o12 = wkp.tile([P, 8, P], F32, tag="o12", name=f"o12{b}")
                sqs = wkp.tile([P, P], F32, tag="sqs", name=f"sqs{b}")
                msb = wkp.tile([P, 8, 1], F32, tag="msb", name=f"msb{b}")
                lns = wkp.tile([P, 8, 1], F32, tag="lns", name=f"lns{b}")
                rs = wkp.tile([P, 8, 1], F32, tag="rs", name=f"rs{b}")
                o2 = wkp.tile([P, 8, P], BF16, tag="o2", name=f"o2{b}")
                nc.vector.tensor_scalar_mul(r1n, asb[:, :, 128:129], -lam)
                for q in range(8):
                    nc.vector.tensor_scalar_mul(
                        o12[:, q, :], asb[:, q, 0:P], bsb[:, q, 128:129]
                    )
                    nc.vector.scalar_tensor_tensor(
                        o12[:, q, :], bsb[:, q, 0:P], r1n[:, q, :],
                        o12[:, q, :], op0=MULT, op1=ADD,
                    )
                    nc.vector.tensor_mul(sqs, o12[:, q, :], o12[:, q, :])
                    nc.vector.tensor_reduce(
                        msb[:, q, :], sqs, mybir.AxisListType.X, ADD
                    )
                make_rms_tail(msb, lns, rs)(slice(0, 8))
                for q in range(8):
                    nc.vector.tensor_scalar_mul(
                        o2[:, q, :], o12[:, q, :], rs[:, q, :]
                    )
                return o2

            def tail_block(b, asb, bsb, otT, tail_rot):
                # final block: pair-major rows (row 2q+h = v-head h, tile q)
                # so the rsqrt batches split per pair-pair; o2/tp/proj for
                # pairs 0,1 overlap the second rsqrt batch. DVE/ACT split
                # throughout so the engines pipeline.
                r1n = wkp.tile([P, 8, 1], F32, tag="r1n", name="r1nT")
                o12 = wkp.tile([P, 8, P], F32, tag="o12", name="o12T")
                sqs = wkp.tile([P, P], F32, tag="sqs", name="sqsT")
                msb = wkp.tile([P, 8, 1], F32, tag="msb", name="msbT")
                lns = wkp.tile([P, 8, 1], F32, tag="lns", name="lnsT")
                rs = wkp.tile([P, 8, 1], F32, tag="rs", name="rsT")
                o2 = wkp.tile([P, 8, P], BF16, tag="o2", name="o2T")
                nc.vector.tensor_scalar_mul(r1n, asb[:, :, 128:129], -lam)
                for rr in range(8):
                    q, h = rr // 2, rr % 2
                    r = q + 4 * h
                    nc.vector.tensor_scalar_mul(
                        o12[:, rr, :], asb[:, r, 0:P], bsb[:, r, 128:129]
                    )
                    nc.vector.scalar_tensor_tensor(
                        o12[:, rr, :], bsb[:, r, 0:P], r1n[:, r, :],
                        o12[:, rr, :], op0=MULT, op1=ADD,
                    )
                    nc.scalar.activation(sqs, o12[:, rr, :], SQR,
                                         accum_out=msb[:, rr, :])
                rms = make_rms_tail(msb, lns, rs)
                for half in range(2):
                    rms(slice(4 * half, 4 * half + 4))
                    for q in (2 * half, 2 * half + 1):
                        nc.vector.tensor_scalar_mul(
                            o2[:, 2 * q, :], o12[:, 2 * q, :],
                            rs[:, 2 * q, :]
                        )
                        nc.scalar.activation(o2[:, 2 * q + 1, :],
                                             o12[:, 2 * q + 1, :],
                                             CPY, scale=rs[:, 2 * q + 1, :])
                        emit_tp(o2, 2 * q, otT, 0, q, psum_src=(scp, "s"),
                                act_copy=False)
                        emit_tp(o2, 2 * q + 1, otT, 1, q,
                                psum_src=(scp, "s"), act_copy=True)
                        for c in proj_tile_closures(otT, b * QS + q,
                                                    tail_rot[q % 4],
                                                    split_copies=True):
                            c()

            def emit_tp(o2, r, otT, vh, q, psum_src=None, act_copy=False):
                # transpose o2 row r ([tq,d2] -> [d2,tq]) via PE
                pool, tag = psum_src if psum_src else (scp, "s")
                pt = pool.tile([P, P], BF16, tag=tag, name=f"tp{vh}{q}")
                nc.tensor.transpose(pt, o2[:, r, :], ident)
                if act_copy:
                    nc.scalar.activation(otT[:, vh, q, :], pt, CPY)
                else:
                    nc.vector.tensor_copy(otT[:, vh, q, :], pt)

            def queue_tp_pairs(o2, otT, b):
                # pairs (vh0 q, vh1 q): keeps the 2-slot scores rotation
                # aligned AND completes one proj tile's deps per pair
                for q in range(QS):
                    def f(q=q):
                        emit_tp(o2, q, otT, 0, q)
                        emit_tp(o2, 4 + q, otT, 1, q)
                    tp_fillers.append((650.0, b, f))

            # -------- output projection for one 128-row tq tile ----------
            def proj_tile_closures(otT, t, psum_src, split_copies):
                q = t % QS
                pool, tag = psum_src
                box = {}
                cl = []

                def c_vh0(p):
                    def f():
                        if p == 0:
                            box["yp"] = pool.tile([P, 2, 256], F32, tag=tag,
                                                  name=f"yt{t}")
                            box["ys"] = ysp.tile([P, 4, 2, 256], BF16,
                                                 tag="ysb", name=f"ys{t}")
                        yp = box["yp"]
                        for r in range(2):
                            nc.tensor.matmul(
                                yp[:, r, :],
                                otT[:, 0, q, :],
                                wp[:, 0, 512 * p + 256 * r:
                                   512 * p + 256 * (r + 1)],
                                start=(r == 0),
                                stop=False,
                            )
                    return f

                def c_vh1(p):
                    def f():
                        yp = box["yp"]
                        for r in range(2):
                            nc.tensor.matmul(
                                yp[:, r, :],
                                otT[:, 1, q, :],
                                wp[:, 1, 512 * p + 256 * r:
                                   512 * p + 256 * (r + 1)],
                                start=False,
                                stop=(r == 1),
                            )
                        if split_copies:
                            nc.vector.tensor_copy(box["ys"][:, p, 0, :],
                                                  yp[:, 0, :])
                            nc.scalar.activation(box["ys"][:, p, 1, :],
                                                 yp[:, 1, :], CPY)
                            if p == 1:  # stream out the finished half
                                nc.sync.dma_start(
                                    out=y_d[t, :, 0:1024],
                                    in_=box["ys"][:, 0:2])
                        else:
                            nc.vector.tensor_copy(box["ys"][:, p, :, :], yp)
                    return f

                def c_dma():
                    if split_copies:
                        nc.sync.dma_start(out=y_d[t, :, 1024:2048],
                                          in_=box["ys"][:, 2:4])
                    else:
                        nc.sync.dma_start(out=y_d[t], in_=box["ys"])

                for p in range(4):
                    cl.append(c_vh0(p))
                    cl.append(c_vh1(p))
                cl.append(c_dma)
                return cl

            # ---------------- blocks ----------------
            tail_rot = [(ypp, "y"), (accp, "accA"), (accp, "accB"),
                        (accp, "accC")]
            warm_tiles = []  # block-2 proj tiles held back to keep the PE
            # (and HAM) warm during the tail combine's DVE/ACT phase
            for b in range(NBLK):
                last = b == NBLK - 1
                drain_aged(b)
                otT = wkp.tile([P, 2, QS, P], BF16, tag="otT", name=f"otT{b}")
                accs0 = sweep(b, 0)
                asb = save_accs(b, 0, accs0, "asb")
                accs1 = sweep(b, 1)
                bsb = save_accs(b, 1, accs1, "bsb")
                if not last:
                    o2 = combine(b, asb, bsb)
                    queue_tp_pairs(o2, otT, b)
                    for t in range(QS):
                        cls = proj_tile_closures(otT, b * QS + t,
                                                 (ypp, "y"),
                                                 split_copies=False)
                        if b == NBLK - 2 and t >= 2:
                            warm_tiles.append(cls)
                        else:
                            proj_fillers.append((b, cls))
                else:
                    # drain leftovers + held-back tiles first (PE chews
                    # them while DVE/ACT run the tail combine), then the
                    # per-pair pipelined tail.
                    finish_owner()
                    for g in list(prod_groups):
                        ensure(g["key"])
                    while tp_fillers:
                        tp_fillers.popleft()[2]()
                    while proj_fillers:
                        _, cls = proj_fillers.popleft()
                        for c in cls:
                            c()
                    for cls in warm_tiles:
                        for c in cls:
                            c()
                    tail_block(b, asb, bsb, otT, tail_rot)
    nc.finalize()
    return nc


def _core_inputs(x, w_qkv, w_proj, rms_scale):
    """Host-side shard prep: per-core bf16 weight slices + replicated x^T."""
    bf = mybir.dt.np(BF16)
    ident = np.ascontiguousarray(np.eye(P, dtype=np.float32).astype(bf))
    xt = x.reshape(T, C).T  # [C, T]
    xtr = np.ascontiguousarray(
        xt.reshape(KS, P, 4, NCH).transpose(1, 2, 0, 3).astype(bf)
    )
    sv = np.tile(
        rms_scale.astype(np.float32) * np.float32(1.0 - LAMBDA_INIT)
        * np.float32(math.sqrt(D2)), 2
    )  # [256]; sqrt(D2) because the kernel's rsqrt takes the SUM of squares
    maps = []
    for c in range(N_CORES):
        cols = [
            w_qkv[:, 0 * 1024 + c * P:0 * 1024 + (c + 1) * P],  # q1 heads 2c,2c+1
            w_qkv[:, 1 * 1024 + c * P:1 * 1024 + (c + 1) * P],  # q2
            w_qkv[:, 2 * 1024 + c * P:2 * 1024 + (c + 1) * P],  # k1
            w_qkv[:, 3 * 1024 + c * P:3 * 1024 + (c + 1) * P],  # k2
        ]
        wqk = np.stack(cols, axis=0)  # [4, C, 128]
        wqk = np.ascontiguousarray(
            wqk.reshape(4, KS, P, P).transpose(2, 0, 1, 3).astype(bf)
        )
        wv = w_qkv[:, 2 * C + c * 2 * D2:2 * C + (c + 1) * 2 * D2]  # [C, 256]
        wv = np.ascontiguousarray(
            wv.reshape(KS, P, 2 * D2).transpose(1, 0, 2).astype(bf)
        )
        wp = w_proj[c * 2 * D2:(c + 1) * 2 * D2, :] * sv[:, None]  # [256, T]
        wp = np.ascontiguousarray(
            wp.reshape(2, P, T).transpose(1, 0, 2).astype(bf)
        )
        maps.append({"xt": xtr, "wqk": wqk, "wv": wv, "wp": wp, "ident": ident})
    return maps


def kernel(x, w_qkv, w_proj, lambda_q1, lambda_k1, lambda_q2, lambda_k2, rms_scale):
    from concourse.bass_utils import run_bass_kernel_spmd

    x = np.asarray(x, dtype=np.float32)
    w_qkv = np.asarray(w_qkv, dtype=np.float32)
    w_proj = np.asarray(w_proj, dtype=np.float32)
    rms_scale = np.asarray(rms_scale, dtype=np.float32)
    lam1 = np.exp(np.sum(np.asarray(lambda_q1) * np.asarray(lambda_k1), dtype=np.float32))
    lam2 = np.exp(np.sum(np.asarray(lambda_q2) * np.asarray(lambda_k2), dtype=np.float32))
    lam = float(lam1 - lam2 + LAMBDA_INIT)

    nc = build(lam)
    in_maps = _core_inputs(x, w_qkv, w_proj, rms_scale)
    res = run_bass_kernel_spmd(nc, in_maps, core_ids=list(range(N_CORES)))
    y = np.zeros((TT, P, T), np.float32)
    for rmap in res.results:
        y += np.asarray(rmap["y"], np.float32)
    return y.reshape(1, T, C)
